# revision 49
# baseline (speedup 1.0000x reference)
"""BFP-quantized linear kernel for Trainium2, 8-core SPMD.

out = bfp_quantize(input) @ bfp_quantize(weight).T + bias
  input  [8192, 4608] f32, weight [4608, 4608] f32, bias [4608] f32
  BFP: groups of 36 contiguous elements (along rows), shared exponent
  from the group absmax, mantissas truncated toward zero to 8 bits.

Key facts exploited:
  * Quantized values are integers i in [-255, 255] times a power-of-two
    step, so they are EXACTLY representable in bf16 -> the matmul runs
    at bf16 speed with no additional error vs the f32 reference.
  * Truncation is pure int16 bit-math on the HIGH HALF of the fp32
    encoding: the mantissa mask -1 << (d+16) (d = e_absmax - e_x) has
    zero low 16 bits, so masking the hi-16 view gives the bf16 result
    directly (probe-validated exact on HW):
        E    = absmax.hi16 >> 7
        exh  = (x.hi16 >> 7) & 0xFF
        d    = E - exh
        zmn  = (d < 8) * -1          # 0xFFFF or 0
        mask = zmn << d              # 0 << anything = 0, so huge d safe
        q    = x.hi16 & mask         # IS the bf16 encoding
  * Sharding: rows of input (1024/core) and rows of weight (576/core).
    Each core quantizes its weight shard in k-quarters, PE-transposes,
    and AllGathers each quarter early so the 4 AGs pipeline with the
    remaining quantization.
  * Matmul accumulates each psum [o-block 128, n 512] tile in k-quarter
    -major order, so a psum group only needs AllGather quarter q (not
    all 4) to make progress; input transposes fill PE gaps between
    quarter arrivals.  Bias rides the PSUM->SBUF drain on the scalar
    engine.  Host transposes the per-core [4608, 1024] result back.
"""

import numpy as np

import concourse.bass as bass
import concourse.mybir as mybir
import concourse.tile as tile
from concourse import bacc
from concourse import bass_utils
from concourse.masks import make_identity

N_CORES = 8
N_ROWS, K_IN, O_OUT = 8192, 4608, 4608
NSH = N_ROWS // N_CORES   # 1024 input rows per core
OSH = O_OUT // N_CORES    # 576 weight rows per core
GS = 36                   # BFP group size
KT = K_IN // 128          # 36 k tiles
NB = NSH // 128           # 8 n blocks per core
NQ = 4                    # k quarters
QW = K_IN // NQ           # 1152 k per quarter = 9 k-tiles
KTQ = QW // 128           # 9
CHUNK = QW                # quantization chunk = one k-quarter row tile
G = CHUNK // GS           # 32 groups per chunk

F32 = mybir.dt.float32
BF16 = mybir.dt.bfloat16
I16 = mybir.dt.int16

W_TILES = [(i * 128, min(128, OSH - i * 128)) for i in range((OSH + 127) // 128)]


def _hi16(ap):
    """Odd-index int16 view (the high half of each f32 element)."""
    return ap.bitcast(I16).rearrange("p (c two) -> p c two", two=2)[:, :, 1]


def _emit_quant_chunk(nc, pool, src, qdst, rows):
    """Quantize src[:rows, :CHUNK] (f32) into qdst[:rows, :CHUNK] (bf16)."""
    xs = src[:rows, :CHUNK]
    xh = _hi16(xs)

    absmax = pool.tile([128, G], F32, tag="absmax", name="absmax")
    nc.vector.tensor_reduce(
        out=absmax[:rows], in_=xs.rearrange("p (g e) -> p g e", e=GS),
        axis=mybir.AxisListType.X,
        op=mybir.AluOpType.max, apply_absolute_value=True,
    )
    E = pool.tile([128, G], I16, tag="E", name="E")
    nc.vector.tensor_scalar(
        out=E[:rows], in0=_hi16(absmax[:rows]),
        scalar1=7, scalar2=None,
        op0=mybir.AluOpType.logical_shift_right,
    )
    # d = E - exh  (in place: exh tile becomes d)
    d = pool.tile([128, CHUNK], I16, tag="qt1", name="d")
    nc.vector.tensor_scalar(
        out=d[:rows], in0=xh, scalar1=7, scalar2=0xFF,
        op0=mybir.AluOpType.logical_shift_right,
        op1=mybir.AluOpType.bitwise_and,
    )
    nc.vector.tensor_tensor(
        out=d[:rows].rearrange("p (g e) -> p g e", e=GS),
        in0=E[:rows].unsqueeze(-1).broadcast_to([rows, G, GS]),
        in1=d[:rows].rearrange("p (g e) -> p g e", e=GS),
        op=mybir.AluOpType.subtract,
    )
    # zmn = -(d < 8); mask = zmn << d (in place over zmn)
    zmn = pool.tile([128, CHUNK], I16, tag="qt2", name="zmn")
    nc.vector.tensor_scalar(
        out=zmn[:rows], in0=d[:rows], scalar1=8, scalar2=-1,
        op0=mybir.AluOpType.is_lt, op1=mybir.AluOpType.mult,
    )
    nc.vector.tensor_tensor(
        out=zmn[:rows], in0=zmn[:rows], in1=d[:rows],
        op=mybir.AluOpType.logical_shift_left,
    )
    nc.vector.tensor_tensor(
        out=qdst[:rows, :CHUNK].bitcast(I16), in0=xh, in1=zmn[:rows],
        op=mybir.AluOpType.bitwise_and,
    )


def emit_kernel(tc, nc, x_d, w_d, b_d, o_d):
    with (
        tc.tile_pool(name="dram", bufs=1, space="DRAM") as dpool,
        tc.tile_pool(name="consts", bufs=1) as cpool,
        tc.tile_pool(name="stage", bufs=2) as spool,
        tc.tile_pool(name="qtmps", bufs=2) as tpool,
        tc.tile_pool(name="qnat", bufs=3) as qpool,
        tc.tile_pool(name="qxt", bufs=1) as xtpool,
        tc.tile_pool(name="wstream", bufs=37) as wpool,
        tc.tile_pool(name="wfirst", bufs=9) as wfpool,
        tc.tile_pool(name="tstage", bufs=2) as tspool,
        tc.tile_pool(name="outs", bufs=2) as opool,
        tc.tile_pool(name="pmm", bufs=6, space="PSUM") as pmm,
        tc.tile_pool(name="ptp", bufs=2, space="PSUM") as ptp,
    ):
        ident = cpool.tile([128, 128], BF16, name="ident")
        make_identity(nc, ident[:])
        # biasT[p, ob] = bias[ob*128 + p]
        biasT = cpool.tile([128, KT], F32, name="biasT")
        nc.sync.dma_start(out=biasT[:], in_=b_d.rearrange("(o p) -> p o", p=128))

        qw_boun = [
            dpool.tile([QW, OSH], BF16, name=f"qw_boun{q}") for q in range(NQ)
        ]
        qwt_g = [
            dpool.tile(
                [N_CORES * QW, OSH], BF16, addr_space="Shared", name=f"qwt_g{q}"
            )
            for q in range(NQ)
        ]
        qxT = [xtpool.tile([128, NSH], BF16, name=f"qxT{kt}") for kt in range(KT)]

        def w_quarter(q):
            """Quantize + transpose + bounce weight k-quarter q, then AG."""
            for r0, rows in W_TILES:
                wtile = spool.tile([128, CHUNK], F32, tag="stage", name="wtile")
                nc.sync.dma_start(
                    out=wtile[:rows], in_=w_d[r0 : r0 + rows, q * QW : (q + 1) * QW]
                )
                qw = qpool.tile([128, CHUNK], BF16, tag="qnat", name="qw")
                _emit_quant_chunk(nc, tpool, wtile, qw, rows)
                for ktl in range(KTQ):
                    pt = ptp.tile([128, 128], BF16, tag="tp", name="pt")
                    nc.tensor.transpose(
                        pt[:, :rows], qw[:rows, ktl * 128 : (ktl + 1) * 128],
                        ident[:rows, :rows],
                    )
                    st = tspool.tile([128, 128], BF16, tag="ts", name="st")
                    nc.scalar.copy(st[:, :rows], pt[:, :rows])
                    nc.sync.dma_start(
                        out=qw_boun[q][ktl * 128 : (ktl + 1) * 128, r0 : r0 + rows],
                        in_=st[:, :rows],
                    )
            nc.gpsimd.collective_compute(
                "AllGather",
                mybir.AluOpType.bypass,
                replica_groups=[list(range(N_CORES))],
                ins=[qw_boun[q][:].opt()],
                outs=[qwt_g[q][:].opt()],
            )

        def x_chunk(nb, q):
            """Quantize + transpose input rows nb*128.. for k-quarter q."""
            xtile = spool.tile([128, CHUNK], F32, tag="stage", name="xtile")
            nc.sync.dma_start(
                out=xtile[:],
                in_=x_d[nb * 128 : (nb + 1) * 128, q * QW : (q + 1) * QW],
            )
            qx = qpool.tile([128, CHUNK], BF16, tag="qnat", name="qx")
            _emit_quant_chunk(nc, tpool, xtile, qx, 128)
            for ktl in range(KTQ):
                kt = q * KTQ + ktl
                pt = ptp.tile([128, 128], BF16, tag="tp", name="pt")
                nc.tensor.transpose(pt[:], qx[:, ktl * 128 : (ktl + 1) * 128], ident[:])
                nc.scalar.copy(qxT[kt][:, nb * 128 : (nb + 1) * 128], pt[:])

        def load_wq(og, q, wq, pool=None):
            """Load gathered weight tiles for o-group og, k-quarter q."""
            pool = wpool if pool is None else pool
            for ktl in range(KTQ):
                wqt = pool.tile([128, 2 * OSH], BF16, tag="wq", name="wqt")
                for h in range(2):
                    c = 2 * og + h
                    nc.sync.dma_start(
                        out=wqt[:, h * OSH : (h + 1) * OSH],
                        in_=qwt_g[q][c * QW + ktl * 128 : c * QW + (ktl + 1) * 128, :],
                    )
                wq[q * KTQ + ktl] = wqt

        def mm_group_quarter(wq, plist, pss, q, kt_start=0, kt_stop=KT - 1):
            """Emit quarter q's matmuls for psum group plist=[(half, obl)...]."""
            for i, (h, obl) in enumerate(plist):
                for ktl in range(KTQ):
                    kt = q * KTQ + ktl
                    nc.tensor.matmul(
                        pss[i][:],
                        wq[kt][:, obl * 128 : (obl + 1) * 128],
                        qxT[kt][:, h * 512 : (h + 1) * 512],
                        start=(kt == kt_start), stop=(kt == kt_stop),
                    )

        def mm_group_drain(og, plist, pss, with_bias=True, dst=None):
            dst = o_d if dst is None else dst
            for i, (h, obl) in enumerate(plist):
                ob = og * 9 + obl
                ot = opool.tile([128, 512], F32, tag="ot", name="ot")
                if with_bias:
                    nc.scalar.activation(
                        ot[:], pss[i][:],
                        mybir.ActivationFunctionType.Identity,
                        bias=biasT[:, ob : ob + 1], scale=1.0,
                    )
                else:
                    nc.scalar.copy(ot[:], pss[i][:])
                r0 = (ob if dst is o_d else obl) * 128
                nc.sync.dma_start(
                    out=dst[r0 : r0 + 128, h * 512 : (h + 1) * 512], in_=ot[:]
                )

        def mm_group(og, wq, plist):
            pss = [pmm.tile([128, 512], F32, tag="mm", name="ps") for _ in plist]
            for q in range(NQ):
                mm_group_quarter(wq, plist, pss, q)
            mm_group_drain(og, plist, pss)



        # ---------------- ramp: quantization + AG pipeline ----------------
        # All weight quarters first so the 4 AllGathers launch back-to-back
        # at link-limited cadence; input quantization follows on DVE while
        # the AGs are in flight.
        # og0's weight loads wait on the AGs, so they are emitted AFTER all
        # quantization stage DMAs: a parked descriptor would otherwise
        # head-of-line-block later bounce/stage traffic on its DMA queue.
        wq0 = {}
        for q in range(NQ):
            w_quarter(q)
        for q in range(NQ):
            for nb in range(4):
                x_chunk(nb, q)
        load_wq(0, 0, wq0)

        # og0 half0 obl0-3: quarter-streamed in the 4 pmm banks, with the
        # nb4-7 input chunks emitted after the q0 batch so their PE
        # transposes fill the AllGather arrival gaps.
        # og0 half0 obl0-5: quarter-streamed in the 6 pmm banks, with the
        # nb4-7 input chunks emitted after the q0 batch so their PE
        # transposes fill the AllGather arrival gaps.
        gA = [(0, obl) for obl in range(6)]
        psA = [pmm.tile([128, 512], F32, tag="mm", name="ps") for _ in gA]
        mm_group_quarter(wq0, gA, psA, 0)
        for nb in range(4, NB):
            for q in range(NQ):
                x_chunk(nb, q)
        # og1's quarter-0 tiles prestage into the dedicated wfirst pool
        # during the AllGather window (they only depend on AG0), removing
        # the load-wait at the og0->og1 transition.
        wq1 = {}
        for q in range(1, NQ):
            load_wq(0, q, wq0)
            mm_group_quarter(wq0, gA, psA, q)
            if q == 1:
                load_wq(1, 0, wq1, pool=wfpool)
        mm_group_drain(0, gA, psA)

        # og0 half0, group B (obl 6-8)
        mm_group(0, wq0, [(0, obl) for obl in range(6, 9)])

        # og1..og3: everything available; 3 groups of <=6 psum tiles
        for og in range(1, 4):
            wq = {}
            if og == 1:
                wq.update(wq1)
                qs = range(1, NQ)
            else:
                qs = range(NQ)
            for q in qs:
                load_wq(og, q, wq)
            mm_group(og, wq, [(0, obl) for obl in range(6)])
            mm_group(og, wq, [(0, 6), (0, 7), (0, 8), (1, 0), (1, 1), (1, 2)])
            mm_group(og, wq, [(1, obl) for obl in range(3, 9)])

        # og0 half1 (deferred; reload og0's weight tiles)
        wq0b = {}
        for q in range(NQ):
            load_wq(0, q, wq0b)
        mm_group(0, wq0b, [(1, obl) for obl in range(6)])
        mm_group(0, wq0b, [(1, obl) for obl in range(6, 9)])


_CACHED_NC = None


def _build():
    global _CACHED_NC
    if _CACHED_NC is not None:
        return _CACHED_NC
    nc = bacc.Bacc(
        "TRN2", target_bir_lowering=False, debug=False, num_devices=N_CORES
    )
    x_d = nc.dram_tensor("x", [NSH, K_IN], F32, kind="ExternalInput").ap()
    w_d = nc.dram_tensor("w", [OSH, K_IN], F32, kind="ExternalInput").ap()
    b_d = nc.dram_tensor("b", [O_OUT], F32, kind="ExternalInput").ap()
    o_d = nc.dram_tensor("o", [O_OUT, NSH], F32, kind="ExternalOutput").ap()
    with tile.TileContext(nc) as tc:
        emit_kernel(tc, nc, x_d, w_d, b_d, o_d)
    nc.compile()
    _CACHED_NC = nc
    return nc


def _ensure_axon_hooks_importable():
    # bass_utils imports antenv.axon_hooks when tracing is requested; the
    # slim agent image lacks it. Provide a no-op so a stray BASS_TRACE env
    # degrades to "no trace" instead of crashing.
    import sys
    import types

    if "antenv.axon_hooks" not in sys.modules:
        try:
            import antenv.axon_hooks  # noqa: F401
        except ImportError:
            mod = types.ModuleType("antenv.axon_hooks")
            mod.get_axon_ntff_profile_hook = lambda: None
            mod.set_axon_ntff_profile_hook = lambda h: None
            sys.modules["antenv.axon_hooks"] = mod


def run_on_hw(input, weight, bias, trace=False):
    _ensure_axon_hooks_importable()
    nc = _build()
    in_maps = []
    for c in range(N_CORES):
        in_maps.append(
            {
                "x": np.ascontiguousarray(input[c * NSH : (c + 1) * NSH]),
                "w": np.ascontiguousarray(weight[c * OSH : (c + 1) * OSH]),
                "b": np.ascontiguousarray(bias),
            }
        )
    res = bass_utils.run_bass_kernel_spmd(
        nc, in_maps, core_ids=list(range(N_CORES)), trace=trace
    )
    out = np.empty((N_ROWS, O_OUT), dtype=np.float32)
    for c in range(N_CORES):
        out[c * NSH : (c + 1) * NSH] = res.results[c]["o"].T
    return out, res


def kernel(input, weight, bias):
    out, _ = run_on_hw(
        np.asarray(input, dtype=np.float32),
        np.asarray(weight, dtype=np.float32),
        np.asarray(bias, dtype=np.float32),
    )
    return out


# revision 50
# speedup vs baseline: 1.1177x; 1.1177x over previous
"""BFP-quantized linear kernel for Trainium2, 8-core SPMD.

out = bfp_quantize(input) @ bfp_quantize(weight).T + bias
  input  [8192, 4608] f32, weight [4608, 4608] f32, bias [4608] f32
  BFP: groups of 36 contiguous elements (along rows), shared exponent
  from the group absmax, mantissas truncated toward zero to 8 bits.

Key facts exploited:
  * Quantized values are integers i in [-255, 255] times a power-of-two
    step, so they are EXACTLY representable in bf16 -> the matmul runs
    at bf16 speed with no additional error vs the f32 reference.
  * Truncation is pure int16 bit-math on the HIGH HALF of the fp32
    encoding: the mantissa mask -1 << (d+16) (d = e_absmax - e_x) has
    zero low 16 bits, so masking the hi-16 view gives the bf16 result
    directly (probe-validated exact on HW):
        E    = absmax.hi16 >> 7
        exh  = (x.hi16 >> 7) & 0xFF
        d    = E - exh
        zmn  = (d < 8) * -1          # 0xFFFF or 0
        mask = zmn << d              # 0 << anything = 0, so huge d safe
        q    = x.hi16 & mask         # IS the bf16 encoding
  * Sharding: rows of input (1024/core) and rows of weight (576/core).
    Each core quantizes its weight shard in k-quarters, PE-transposes,
    and AllGathers each quarter early so the 4 AGs pipeline with the
    remaining quantization.
  * Matmul accumulates each psum [o-block 128, n 512] tile in k-quarter
    -major order, so a psum group only needs AllGather quarter q (not
    all 4) to make progress; input transposes fill PE gaps between
    quarter arrivals.  Bias rides the PSUM->SBUF drain on the scalar
    engine.  Host transposes the per-core [4608, 1024] result back.
"""

import numpy as np

import concourse.bass as bass
import concourse.mybir as mybir
import concourse.tile as tile
from concourse import bacc
from concourse import bass_utils
from concourse.masks import make_identity

N_CORES = 8
N_ROWS, K_IN, O_OUT = 8192, 4608, 4608
NSH = N_ROWS // N_CORES   # 1024 input rows per core
OSH = O_OUT // N_CORES    # 576 weight rows per core
GS = 36                   # BFP group size
KT = K_IN // 128          # 36 k tiles
NB = NSH // 128           # 8 n blocks per core
NQ = 4                    # k quarters
QW = K_IN // NQ           # 1152 k per quarter = 9 k-tiles
KTQ = QW // 128           # 9
CHUNK = QW                # quantization chunk = one k-quarter row tile
G = CHUNK // GS           # 32 groups per chunk

F32 = mybir.dt.float32
BF16 = mybir.dt.bfloat16
I16 = mybir.dt.int16

W_TILES = [(i * 128, min(128, OSH - i * 128)) for i in range((OSH + 127) // 128)]


def _hi16(ap):
    """Odd-index int16 view (the high half of each f32 element)."""
    return ap.bitcast(I16).rearrange("p (c two) -> p c two", two=2)[:, :, 1]


def _emit_quant_chunk(nc, pool, src, qdst, rows):
    """Quantize src[:rows, :CHUNK] (f32) into qdst[:rows, :CHUNK] (bf16)."""
    xs = src[:rows, :CHUNK]
    xh = _hi16(xs)

    absmax = pool.tile([128, G], F32, tag="absmax", name="absmax")
    nc.vector.tensor_reduce(
        out=absmax[:rows], in_=xs.rearrange("p (g e) -> p g e", e=GS),
        axis=mybir.AxisListType.X,
        op=mybir.AluOpType.max, apply_absolute_value=True,
    )
    E = pool.tile([128, G], I16, tag="E", name="E")
    nc.vector.tensor_scalar(
        out=E[:rows], in0=_hi16(absmax[:rows]),
        scalar1=7, scalar2=None,
        op0=mybir.AluOpType.logical_shift_right,
    )
    # d = E - exh  (in place: exh tile becomes d)
    d = pool.tile([128, CHUNK], I16, tag="qt1", name="d")
    nc.vector.tensor_scalar(
        out=d[:rows], in0=xh, scalar1=7, scalar2=0xFF,
        op0=mybir.AluOpType.logical_shift_right,
        op1=mybir.AluOpType.bitwise_and,
    )
    nc.vector.tensor_tensor(
        out=d[:rows].rearrange("p (g e) -> p g e", e=GS),
        in0=E[:rows].unsqueeze(-1).broadcast_to([rows, G, GS]),
        in1=d[:rows].rearrange("p (g e) -> p g e", e=GS),
        op=mybir.AluOpType.subtract,
    )
    # zmn = -(d < 8); mask = zmn << d (in place over zmn)
    zmn = pool.tile([128, CHUNK], I16, tag="qt2", name="zmn")
    nc.vector.tensor_scalar(
        out=zmn[:rows], in0=d[:rows], scalar1=8, scalar2=-1,
        op0=mybir.AluOpType.is_lt, op1=mybir.AluOpType.mult,
    )
    nc.vector.tensor_tensor(
        out=zmn[:rows], in0=zmn[:rows], in1=d[:rows],
        op=mybir.AluOpType.logical_shift_left,
    )
    nc.vector.tensor_tensor(
        out=qdst[:rows, :CHUNK].bitcast(I16), in0=xh, in1=zmn[:rows],
        op=mybir.AluOpType.bitwise_and,
    )


def emit_kernel(tc, nc, x_d, w_d, b_d, o_d):
    with (
        tc.tile_pool(name="dram", bufs=1, space="DRAM") as dpool,
        tc.tile_pool(name="consts", bufs=1) as cpool,
        tc.tile_pool(name="stage", bufs=3) as spool,
        tc.tile_pool(name="qtmps", bufs=2) as tpool,
        tc.tile_pool(name="qnat", bufs=3) as qpool,
        tc.tile_pool(name="qxt", bufs=1) as xtpool,
        tc.tile_pool(name="wstream", bufs=38) as wpool,
        tc.tile_pool(name="tstage", bufs=4) as tspool,
        tc.tile_pool(name="outs", bufs=3) as opool,
        tc.tile_pool(name="pmm", bufs=6, space="PSUM") as pmm,
        tc.tile_pool(name="ptp", bufs=2, space="PSUM") as ptp,
    ):
        ident = cpool.tile([128, 128], BF16, name="ident")
        make_identity(nc, ident[:])
        # biasT[p, ob] = bias[ob*128 + p]
        biasT = cpool.tile([128, KT], F32, name="biasT")
        nc.sync.dma_start(out=biasT[:], in_=b_d.rearrange("(o p) -> p o", p=128))

        qw_boun = [
            dpool.tile([QW, OSH], BF16, name=f"qw_boun{q}") for q in range(NQ)
        ]
        qwt_g = [
            dpool.tile(
                [N_CORES * QW, OSH], BF16, addr_space="Shared", name=f"qwt_g{q}"
            )
            for q in range(NQ)
        ]
        qxT = [xtpool.tile([128, NSH], BF16, name=f"qxT{kt}") for kt in range(KT)]

        def w_quarter(q):
            """Quantize + transpose + bounce weight k-quarter q, then AG."""
            for r0, rows in W_TILES:
                wtile = spool.tile([128, CHUNK], F32, tag="stage", name="wtile")
                nc.sync.dma_start(
                    out=wtile[:rows], in_=w_d[r0 : r0 + rows, q * QW : (q + 1) * QW]
                )
                qw = qpool.tile([128, CHUNK], BF16, tag="qnat", name="qw")
                _emit_quant_chunk(nc, tpool, wtile, qw, rows)
                for ktl in range(KTQ):
                    pt = ptp.tile([128, 128], BF16, tag="tp", name="pt")
                    nc.tensor.transpose(
                        pt[:, :rows], qw[:rows, ktl * 128 : (ktl + 1) * 128],
                        ident[:rows, :rows],
                    )
                    st = tspool.tile([128, 128], BF16, tag="ts", name="st")
                    nc.scalar.copy(st[:, :rows], pt[:, :rows])
                    nc.sync.dma_start(
                        out=qw_boun[q][ktl * 128 : (ktl + 1) * 128, r0 : r0 + rows],
                        in_=st[:, :rows],
                    )
            nc.gpsimd.collective_compute(
                "AllGather",
                mybir.AluOpType.bypass,
                replica_groups=[list(range(N_CORES))],
                ins=[qw_boun[q][:].opt()],
                outs=[qwt_g[q][:].opt()],
            )

        def x_chunk(nb, q):
            """Quantize + transpose input rows nb*128.. for k-quarter q."""
            xtile = spool.tile([128, CHUNK], F32, tag="stage", name="xtile")
            nc.sync.dma_start(
                out=xtile[:],
                in_=x_d[nb * 128 : (nb + 1) * 128, q * QW : (q + 1) * QW],
            )
            qx = qpool.tile([128, CHUNK], BF16, tag="qnat", name="qx")
            _emit_quant_chunk(nc, tpool, xtile, qx, 128)
            for ktl in range(KTQ):
                kt = q * KTQ + ktl
                pt = ptp.tile([128, 128], BF16, tag="tp", name="pt")
                nc.tensor.transpose(pt[:], qx[:, ktl * 128 : (ktl + 1) * 128], ident[:])
                nc.scalar.copy(qxT[kt][:, nb * 128 : (nb + 1) * 128], pt[:])

        def load_wq(og, q, wq):
            """Load gathered weight tiles for o-group og, k-quarter q."""
            for ktl in range(KTQ):
                wqt = wpool.tile([128, 2 * OSH], BF16, tag="wq", name="wqt")
                for h in range(2):
                    c = 2 * og + h
                    nc.sync.dma_start(
                        out=wqt[:, h * OSH : (h + 1) * OSH],
                        in_=qwt_g[q][c * QW + ktl * 128 : c * QW + (ktl + 1) * 128, :],
                    )
                wq[q * KTQ + ktl] = wqt

        def mm_group_quarter(wq, plist, pss, q, kt_start=0, kt_stop=KT - 1):
            """Emit quarter q's matmuls for psum group plist=[(half, obl)...]."""
            for i, (h, obl) in enumerate(plist):
                for ktl in range(KTQ):
                    kt = q * KTQ + ktl
                    nc.tensor.matmul(
                        pss[i][:],
                        wq[kt][:, obl * 128 : (obl + 1) * 128],
                        qxT[kt][:, h * 512 : (h + 1) * 512],
                        start=(kt == kt_start), stop=(kt == kt_stop),
                    )

        def mm_group_drain(og, plist, pss, with_bias=True, dst=None):
            dst = o_d if dst is None else dst
            for i, (h, obl) in enumerate(plist):
                ob = og * 9 + obl
                ot = opool.tile([128, 512], F32, tag="ot", name="ot")
                if with_bias:
                    nc.scalar.activation(
                        ot[:], pss[i][:],
                        mybir.ActivationFunctionType.Identity,
                        bias=biasT[:, ob : ob + 1], scale=1.0,
                    )
                else:
                    nc.scalar.copy(ot[:], pss[i][:])
                r0 = (ob if dst is o_d else obl) * 128
                nc.sync.dma_start(
                    out=dst[r0 : r0 + 128, h * 512 : (h + 1) * 512], in_=ot[:]
                )

        def mm_group(og, wq, plist):
            pss = [pmm.tile([128, 512], F32, tag="mm", name="ps") for _ in plist]
            for q in range(NQ):
                mm_group_quarter(wq, plist, pss, q)
            mm_group_drain(og, plist, pss)



        # ---------------- ramp: quantization + AG pipeline ----------------
        # All weight quarters first so the 4 AllGathers launch back-to-back
        # at link-limited cadence; input quantization follows on DVE while
        # the AGs are in flight.
        # og0's weight loads wait on the AGs, so they are emitted AFTER all
        # quantization stage DMAs: a parked descriptor would otherwise
        # head-of-line-block later bounce/stage traffic on its DMA queue.
        wq0 = {}
        for q in range(NQ):
            w_quarter(q)
        for q in range(NQ):
            for nb in range(4):
                x_chunk(nb, q)
        load_wq(0, 0, wq0)

        # og0 half0 obl0-3: quarter-streamed in the 4 pmm banks, with the
        # nb4-7 input chunks emitted after the q0 batch so their PE
        # transposes fill the AllGather arrival gaps.
        # og0 half0 obl0-5: quarter-streamed in the 6 pmm banks, with the
        # nb4-7 input chunks emitted after the q0 batch so their PE
        # transposes fill the AllGather arrival gaps.
        gA = [(0, obl) for obl in range(6)]
        psA = [pmm.tile([128, 512], F32, tag="mm", name="ps") for _ in gA]
        mm_group_quarter(wq0, gA, psA, 0)
        for nb in range(4, NB):
            for q in range(NQ):
                x_chunk(nb, q)
        for q in range(1, NQ):
            load_wq(0, q, wq0)
            mm_group_quarter(wq0, gA, psA, q)
        mm_group_drain(0, gA, psA)

        # og0 half0, group B (obl 6-8)
        mm_group(0, wq0, [(0, obl) for obl in range(6, 9)])

        # og1..og3: everything available; 3 groups of <=6 psum tiles
        for og in range(1, 4):
            wq = {}
            for q in range(NQ):
                load_wq(og, q, wq)
            mm_group(og, wq, [(0, obl) for obl in range(6)])
            mm_group(og, wq, [(0, 6), (0, 7), (0, 8), (1, 0), (1, 1), (1, 2)])
            mm_group(og, wq, [(1, obl) for obl in range(3, 9)])

        # og0 half1 (deferred; reload og0's weight tiles)
        wq0b = {}
        for q in range(NQ):
            load_wq(0, q, wq0b)
        mm_group(0, wq0b, [(1, obl) for obl in range(6)])
        mm_group(0, wq0b, [(1, obl) for obl in range(6, 9)])


_CACHED_NC = None


def _build():
    global _CACHED_NC
    if _CACHED_NC is not None:
        return _CACHED_NC
    nc = bacc.Bacc(
        "TRN2", target_bir_lowering=False, debug=False, num_devices=N_CORES
    )
    x_d = nc.dram_tensor("x", [NSH, K_IN], F32, kind="ExternalInput").ap()
    w_d = nc.dram_tensor("w", [OSH, K_IN], F32, kind="ExternalInput").ap()
    b_d = nc.dram_tensor("b", [O_OUT], F32, kind="ExternalInput").ap()
    o_d = nc.dram_tensor("o", [O_OUT, NSH], F32, kind="ExternalOutput").ap()
    with tile.TileContext(nc) as tc:
        emit_kernel(tc, nc, x_d, w_d, b_d, o_d)
    nc.compile()
    _CACHED_NC = nc
    return nc


def _ensure_axon_hooks_importable():
    # bass_utils imports antenv.axon_hooks when tracing is requested; the
    # slim agent image lacks it. Provide a no-op so a stray BASS_TRACE env
    # degrades to "no trace" instead of crashing.
    import sys
    import types

    if "antenv.axon_hooks" not in sys.modules:
        try:
            import antenv.axon_hooks  # noqa: F401
        except ImportError:
            mod = types.ModuleType("antenv.axon_hooks")
            mod.get_axon_ntff_profile_hook = lambda: None
            mod.set_axon_ntff_profile_hook = lambda h: None
            sys.modules["antenv.axon_hooks"] = mod


def run_on_hw(input, weight, bias, trace=False):
    _ensure_axon_hooks_importable()
    nc = _build()
    in_maps = []
    for c in range(N_CORES):
        in_maps.append(
            {
                "x": np.ascontiguousarray(input[c * NSH : (c + 1) * NSH]),
                "w": np.ascontiguousarray(weight[c * OSH : (c + 1) * OSH]),
                "b": np.ascontiguousarray(bias),
            }
        )
    res = bass_utils.run_bass_kernel_spmd(
        nc, in_maps, core_ids=list(range(N_CORES)), trace=trace
    )
    out = np.empty((N_ROWS, O_OUT), dtype=np.float32)
    for c in range(N_CORES):
        out[c * NSH : (c + 1) * NSH] = res.results[c]["o"].T
    return out, res


def kernel(input, weight, bias):
    out, _ = run_on_hw(
        np.asarray(input, dtype=np.float32),
        np.asarray(weight, dtype=np.float32),
        np.asarray(bias, dtype=np.float32),
    )
    return out


# revision 51
# speedup vs baseline: 1.1365x; 1.0168x over previous
"""BFP-quantized linear kernel for Trainium2, 8-core SPMD.

out = bfp_quantize(input) @ bfp_quantize(weight).T + bias
  input  [8192, 4608] f32, weight [4608, 4608] f32, bias [4608] f32
  BFP: groups of 36 contiguous elements (along rows), shared exponent
  from the group absmax, mantissas truncated toward zero to 8 bits.

Key facts exploited:
  * Quantized values are integers i in [-255, 255] times a power-of-two
    step, so they are EXACTLY representable in bf16 -> the matmul runs
    at bf16 speed with no additional error vs the f32 reference.
  * Truncation is pure int16 bit-math on the HIGH HALF of the fp32
    encoding: the mantissa mask -1 << (d+16) (d = e_absmax - e_x) has
    zero low 16 bits, so masking the hi-16 view gives the bf16 result
    directly (probe-validated exact on HW):
        E    = absmax.hi16 >> 7
        exh  = (x.hi16 >> 7) & 0xFF
        d    = E - exh
        zmn  = (d < 8) * -1          # 0xFFFF or 0
        mask = zmn << d              # 0 << anything = 0, so huge d safe
        q    = x.hi16 & mask         # IS the bf16 encoding
  * Sharding: rows of input (1024/core) and rows of weight (576/core).
    Each core quantizes its weight shard in k-quarters, PE-transposes,
    and AllGathers each quarter early so the 4 AGs pipeline with the
    remaining quantization.
  * Matmul accumulates each psum [o-block 128, n 512] tile in k-quarter
    -major order, so a psum group only needs AllGather quarter q (not
    all 4) to make progress; input transposes fill PE gaps between
    quarter arrivals.  Bias rides the PSUM->SBUF drain on the scalar
    engine.  Host transposes the per-core [4608, 1024] result back.
"""

import numpy as np

import concourse.bass as bass
import concourse.mybir as mybir
import concourse.tile as tile
from concourse import bacc
from concourse import bass_utils
from concourse.masks import make_identity

N_CORES = 8
N_ROWS, K_IN, O_OUT = 8192, 4608, 4608
NSH = N_ROWS // N_CORES   # 1024 input rows per core
OSH = O_OUT // N_CORES    # 576 weight rows per core
GS = 36                   # BFP group size
KT = K_IN // 128          # 36 k tiles
NB = NSH // 128           # 8 n blocks per core
NQ = 4                    # k quarters
QW = K_IN // NQ           # 1152 k per quarter = 9 k-tiles
KTQ = QW // 128           # 9
CHUNK = QW                # quantization chunk = one k-quarter row tile
G = CHUNK // GS           # 32 groups per chunk

F32 = mybir.dt.float32
BF16 = mybir.dt.bfloat16
I16 = mybir.dt.int16

W_TILES = [(i * 128, min(128, OSH - i * 128)) for i in range((OSH + 127) // 128)]


def _hi16(ap):
    """Odd-index int16 view (the high half of each f32 element)."""
    return ap.bitcast(I16).rearrange("p (c two) -> p c two", two=2)[:, :, 1]


def _emit_quant_chunk(nc, pool, src, qdst, rows):
    """Quantize src[:rows, :CHUNK] (f32) into qdst[:rows, :CHUNK] (bf16)."""
    xs = src[:rows, :CHUNK]
    xh = _hi16(xs)

    absmax = pool.tile([128, G], F32, tag="absmax", name="absmax")
    nc.vector.tensor_reduce(
        out=absmax[:rows], in_=xs.rearrange("p (g e) -> p g e", e=GS),
        axis=mybir.AxisListType.X,
        op=mybir.AluOpType.max, apply_absolute_value=True,
    )
    E = pool.tile([128, G], I16, tag="E", name="E")
    nc.vector.tensor_scalar(
        out=E[:rows], in0=_hi16(absmax[:rows]),
        scalar1=7, scalar2=None,
        op0=mybir.AluOpType.logical_shift_right,
    )
    # d = E - exh  (in place: exh tile becomes d)
    d = pool.tile([128, CHUNK], I16, tag="qt1", name="d")
    nc.vector.tensor_scalar(
        out=d[:rows], in0=xh, scalar1=7, scalar2=0xFF,
        op0=mybir.AluOpType.logical_shift_right,
        op1=mybir.AluOpType.bitwise_and,
    )
    nc.vector.tensor_tensor(
        out=d[:rows].rearrange("p (g e) -> p g e", e=GS),
        in0=E[:rows].unsqueeze(-1).broadcast_to([rows, G, GS]),
        in1=d[:rows].rearrange("p (g e) -> p g e", e=GS),
        op=mybir.AluOpType.subtract,
    )
    # zmn = -(d < 8); mask = zmn << d (in place over zmn)
    zmn = pool.tile([128, CHUNK], I16, tag="qt2", name="zmn")
    nc.vector.tensor_scalar(
        out=zmn[:rows], in0=d[:rows], scalar1=8, scalar2=-1,
        op0=mybir.AluOpType.is_lt, op1=mybir.AluOpType.mult,
    )
    nc.vector.tensor_tensor(
        out=zmn[:rows], in0=zmn[:rows], in1=d[:rows],
        op=mybir.AluOpType.logical_shift_left,
    )
    nc.vector.tensor_tensor(
        out=qdst[:rows, :CHUNK].bitcast(I16), in0=xh, in1=zmn[:rows],
        op=mybir.AluOpType.bitwise_and,
    )


def emit_kernel(tc, nc, x_d, w_d, b_d, o_d):
    with (
        tc.tile_pool(name="dram", bufs=1, space="DRAM") as dpool,
        tc.tile_pool(name="consts", bufs=1) as cpool,
        tc.tile_pool(name="stage", bufs=3) as spool,
        tc.tile_pool(name="qtmps", bufs=3) as tpool,
        tc.tile_pool(name="qnat", bufs=3) as qpool,
        tc.tile_pool(name="qxt", bufs=1) as xtpool,
        tc.tile_pool(name="wstream", bufs=38) as wpool,
        tc.tile_pool(name="tstage", bufs=4) as tspool,
        tc.tile_pool(name="outs", bufs=3) as opool,
        tc.tile_pool(name="pmm", bufs=6, space="PSUM") as pmm,
        tc.tile_pool(name="ptp", bufs=2, space="PSUM") as ptp,
    ):
        ident = cpool.tile([128, 128], BF16, name="ident")
        make_identity(nc, ident[:])
        # biasT[p, ob] = bias[ob*128 + p]
        biasT = cpool.tile([128, KT], F32, name="biasT")
        nc.sync.dma_start(out=biasT[:], in_=b_d.rearrange("(o p) -> p o", p=128))

        qw_boun = [
            dpool.tile([QW, OSH], BF16, name=f"qw_boun{q}") for q in range(NQ)
        ]
        qwt_g = [
            dpool.tile(
                [N_CORES * QW, OSH], BF16, addr_space="Shared", name=f"qwt_g{q}"
            )
            for q in range(NQ)
        ]
        qxT = [xtpool.tile([128, NSH], BF16, name=f"qxT{kt}") for kt in range(KT)]

        def w_quarter(q):
            """Quantize + transpose + bounce weight k-quarter q, then AG."""
            for r0, rows in W_TILES:
                wtile = spool.tile([128, CHUNK], F32, tag="stage", name="wtile")
                nc.sync.dma_start(
                    out=wtile[:rows], in_=w_d[r0 : r0 + rows, q * QW : (q + 1) * QW]
                )
                qw = qpool.tile([128, CHUNK], BF16, tag="qnat", name="qw")
                _emit_quant_chunk(nc, tpool, wtile, qw, rows)
                for ktl in range(KTQ):
                    pt = ptp.tile([128, 128], BF16, tag="tp", name="pt")
                    nc.tensor.transpose(
                        pt[:, :rows], qw[:rows, ktl * 128 : (ktl + 1) * 128],
                        ident[:rows, :rows],
                    )
                    st = tspool.tile([128, 128], BF16, tag="ts", name="st")
                    nc.scalar.copy(st[:, :rows], pt[:, :rows])
                    nc.sync.dma_start(
                        out=qw_boun[q][ktl * 128 : (ktl + 1) * 128, r0 : r0 + rows],
                        in_=st[:, :rows],
                    )
            nc.gpsimd.collective_compute(
                "AllGather",
                mybir.AluOpType.bypass,
                replica_groups=[list(range(N_CORES))],
                ins=[qw_boun[q][:].opt()],
                outs=[qwt_g[q][:].opt()],
            )

        def x_chunk(nb, q):
            """Quantize + transpose input rows nb*128.. for k-quarter q."""
            xtile = spool.tile([128, CHUNK], F32, tag="stage", name="xtile")
            nc.sync.dma_start(
                out=xtile[:],
                in_=x_d[nb * 128 : (nb + 1) * 128, q * QW : (q + 1) * QW],
            )
            qx = qpool.tile([128, CHUNK], BF16, tag="qnat", name="qx")
            _emit_quant_chunk(nc, tpool, xtile, qx, 128)
            for ktl in range(KTQ):
                kt = q * KTQ + ktl
                pt = ptp.tile([128, 128], BF16, tag="tp", name="pt")
                nc.tensor.transpose(pt[:], qx[:, ktl * 128 : (ktl + 1) * 128], ident[:])
                nc.scalar.copy(qxT[kt][:, nb * 128 : (nb + 1) * 128], pt[:])

        def load_wq(og, q, wq):
            """Load gathered weight tiles for o-group og, k-quarter q."""
            for ktl in range(KTQ):
                wqt = wpool.tile([128, 2 * OSH], BF16, tag="wq", name="wqt")
                for h in range(2):
                    c = 2 * og + h
                    nc.sync.dma_start(
                        out=wqt[:, h * OSH : (h + 1) * OSH],
                        in_=qwt_g[q][c * QW + ktl * 128 : c * QW + (ktl + 1) * 128, :],
                    )
                wq[q * KTQ + ktl] = wqt

        def mm_group_quarter(wq, plist, pss, q, kt_start=0, kt_stop=KT - 1):
            """Emit quarter q's matmuls for psum group plist=[(half, obl)...]."""
            for i, (h, obl) in enumerate(plist):
                for ktl in range(KTQ):
                    kt = q * KTQ + ktl
                    nc.tensor.matmul(
                        pss[i][:],
                        wq[kt][:, obl * 128 : (obl + 1) * 128],
                        qxT[kt][:, h * 512 : (h + 1) * 512],
                        start=(kt == kt_start), stop=(kt == kt_stop),
                    )

        def mm_group_drain(og, plist, pss, with_bias=True, dst=None):
            dst = o_d if dst is None else dst
            for i, (h, obl) in enumerate(plist):
                ob = og * 9 + obl
                ot = opool.tile([128, 512], F32, tag="ot", name="ot")
                if with_bias:
                    nc.scalar.activation(
                        ot[:], pss[i][:],
                        mybir.ActivationFunctionType.Identity,
                        bias=biasT[:, ob : ob + 1], scale=1.0,
                    )
                else:
                    nc.scalar.copy(ot[:], pss[i][:])
                r0 = (ob if dst is o_d else obl) * 128
                nc.sync.dma_start(
                    out=dst[r0 : r0 + 128, h * 512 : (h + 1) * 512], in_=ot[:]
                )

        def mm_group(og, wq, plist):
            pss = [pmm.tile([128, 512], F32, tag="mm", name="ps") for _ in plist]
            for q in range(NQ):
                mm_group_quarter(wq, plist, pss, q)
            mm_group_drain(og, plist, pss)



        # ---------------- ramp: quantization + AG pipeline ----------------
        # All weight quarters first so the 4 AllGathers launch back-to-back
        # at link-limited cadence; input quantization follows on DVE while
        # the AGs are in flight.
        # og0's weight loads wait on the AGs, so they are emitted AFTER all
        # quantization stage DMAs: a parked descriptor would otherwise
        # head-of-line-block later bounce/stage traffic on its DMA queue.
        wq0 = {}
        for q in range(NQ):
            w_quarter(q)
        for q in range(NQ):
            for nb in range(4):
                x_chunk(nb, q)
        load_wq(0, 0, wq0)

        # og0 half0 obl0-3: quarter-streamed in the 4 pmm banks, with the
        # nb4-7 input chunks emitted after the q0 batch so their PE
        # transposes fill the AllGather arrival gaps.
        # og0 half0 obl0-5: quarter-streamed in the 6 pmm banks, with the
        # nb4-7 input chunks emitted after the q0 batch so their PE
        # transposes fill the AllGather arrival gaps.
        gA = [(0, obl) for obl in range(6)]
        psA = [pmm.tile([128, 512], F32, tag="mm", name="ps") for _ in gA]
        mm_group_quarter(wq0, gA, psA, 0)
        for nb in range(4, NB):
            for q in range(NQ):
                x_chunk(nb, q)
        for q in range(1, NQ):
            load_wq(0, q, wq0)
            mm_group_quarter(wq0, gA, psA, q)
        mm_group_drain(0, gA, psA)

        # og0 half0, group B (obl 6-8)
        mm_group(0, wq0, [(0, obl) for obl in range(6, 9)])

        # og1..og3: everything available; 3 groups of <=6 psum tiles
        for og in range(1, 4):
            wq = {}
            for q in range(NQ):
                load_wq(og, q, wq)
            mm_group(og, wq, [(0, obl) for obl in range(6)])
            mm_group(og, wq, [(0, 6), (0, 7), (0, 8), (1, 0), (1, 1), (1, 2)])
            mm_group(og, wq, [(1, obl) for obl in range(3, 9)])

        # og0 half1 (deferred; reload og0's weight tiles)
        wq0b = {}
        for q in range(NQ):
            load_wq(0, q, wq0b)
        mm_group(0, wq0b, [(1, obl) for obl in range(6)])
        mm_group(0, wq0b, [(1, obl) for obl in range(6, 9)])


_CACHED_NC = None


def _build():
    global _CACHED_NC
    if _CACHED_NC is not None:
        return _CACHED_NC
    nc = bacc.Bacc(
        "TRN2", target_bir_lowering=False, debug=False, num_devices=N_CORES
    )
    x_d = nc.dram_tensor("x", [NSH, K_IN], F32, kind="ExternalInput").ap()
    w_d = nc.dram_tensor("w", [OSH, K_IN], F32, kind="ExternalInput").ap()
    b_d = nc.dram_tensor("b", [O_OUT], F32, kind="ExternalInput").ap()
    o_d = nc.dram_tensor("o", [O_OUT, NSH], F32, kind="ExternalOutput").ap()
    with tile.TileContext(nc) as tc:
        emit_kernel(tc, nc, x_d, w_d, b_d, o_d)
    nc.compile()
    _CACHED_NC = nc
    return nc


def _ensure_axon_hooks_importable():
    # bass_utils imports antenv.axon_hooks when tracing is requested; the
    # slim agent image lacks it. Provide a no-op so a stray BASS_TRACE env
    # degrades to "no trace" instead of crashing.
    import sys
    import types

    if "antenv.axon_hooks" not in sys.modules:
        try:
            import antenv.axon_hooks  # noqa: F401
        except ImportError:
            mod = types.ModuleType("antenv.axon_hooks")
            mod.get_axon_ntff_profile_hook = lambda: None
            mod.set_axon_ntff_profile_hook = lambda h: None
            sys.modules["antenv.axon_hooks"] = mod


def run_on_hw(input, weight, bias, trace=False):
    _ensure_axon_hooks_importable()
    nc = _build()
    in_maps = []
    for c in range(N_CORES):
        in_maps.append(
            {
                "x": np.ascontiguousarray(input[c * NSH : (c + 1) * NSH]),
                "w": np.ascontiguousarray(weight[c * OSH : (c + 1) * OSH]),
                "b": np.ascontiguousarray(bias),
            }
        )
    res = bass_utils.run_bass_kernel_spmd(
        nc, in_maps, core_ids=list(range(N_CORES)), trace=trace
    )
    out = np.empty((N_ROWS, O_OUT), dtype=np.float32)
    for c in range(N_CORES):
        out[c * NSH : (c + 1) * NSH] = res.results[c]["o"].T
    return out, res


def kernel(input, weight, bias):
    out, _ = run_on_hw(
        np.asarray(input, dtype=np.float32),
        np.asarray(weight, dtype=np.float32),
        np.asarray(bias, dtype=np.float32),
    )
    return out
